# revision 78
# baseline (speedup 1.0000x reference)
"""Bass/Trainium2 kernel for DynamicMultiheadAttention (sparse_attention).

Sharding: 8 cores = (batch b) x (query-half of 1024 rows) x (head-half of
4 heads).  Head-parallelism halves the replicated K/V projection work; the
two head-halves produce partial output projections that the host sums.
Each core computes its 4 heads for its (b, n-half) in transposed
orientation: scores sT[m, n] with keys m on partitions, so that
  - the relative-mask bias  -sum_r c[h,r]*M_r[m,n]  is accumulated into
    score PSUM by fp8 DoubleRow matmuls (0.5 PE cycles/row, two mask
    planes per matmul) with diag stationaries; each coefficient is split
    into hi+lo fp8e4 terms (max residual ~7.5e-3),
  - softmax row-sums come free from a ones-column appended to V,
  - attn @ V needs no transposes (pT tiles are directly the stationary
    operand layout).
Key padding is applied by zeroing padded key rows of V and of the
ones-column (exactly equivalent to -inf logits). The row-constant term
scale_h * sum_r w[h,r] = scale_h cancels in softmax and is dropped; the
k-projection bias is softmax-invariant and dropped; the v bias folds
into the output bias (softmax rows sum to 1): bo' = bv @ Wo + bo (added
on the head-half-0 cores only; the host sums the two partial outputs).

Pipelining: two heads share a 2-bank PSUM score tile so one Exp covers
both; the attn@V matmuls run one key-tile behind the score matmuls so PE
never waits on the current Exp; mask/ident loads stream on the Pool
engine's SWDGE path gated behind the critical projection loads; the
softmax reciprocal row is rounded to bf16 on the Pool engine and
broadcast by a K=1 matmul into the evacuated upper partition half of the
even head's o_ps PSUM bank.

Every TPB instruction encoding in this walrus build tolerates only ONE
semaphore wait; a post-pass (_split_matmul_waits) moves extra waits onto
standalone single-wait EventSemaphore instructions inserted before the
offending instruction on the same engine queue.

All matmuls run as float32r (1 PE cycle/row vs 4 for fp32); projection
inputs x/W are bf16 (halves the load bytes; the score/attention path
stays f32r); the mask-bias path is fp8.
"""

import numpy as np
import ml_dtypes
import os

def _B(name, default):
    return int(os.environ.get("KB_" + name, default))

N, B, D = 2048, 2, 512
H, R = 8, 3
C = D // H          # 64
NSH = N // 2        # 1024 query rows per core
NT = NSH // 512     # 2 query tiles per core
HH = H // 2         # 4 heads per core
DH = HH * C         # 256 projected dims per core
NCORES = 8
MT = N // 128       # 16 key tiles

_cache = {}


def _build_program(reps=1):
    import concourse.bass as bass
    import concourse.mybir as mybir
    import concourse.tile as tile
    from contextlib import ExitStack

    f32 = mybir.dt.float32
    f32r = mybir.dt.float32r
    bf16 = mybir.dt.bfloat16
    f8 = mybir.dt.float8e4

    nc = bass.Bass()

    xtq = nc.declare_dram_parameter("xtq", [D, NSH], bf16, isOutput=False)
    xtk = nc.declare_dram_parameter("xtk", [D, N], bf16, isOutput=False)
    xtv = nc.declare_dram_parameter("xtv", [D, N], bf16, isOutput=False)
    masksT = nc.declare_dram_parameter("masksT", [N, 3 * NSH], f8, isOutput=False)
    wq = nc.declare_dram_parameter("wq", [D, DH], bf16, isOutput=False)
    wk = nc.declare_dram_parameter("wk", [D, DH], bf16, isOutput=False)
    wv = nc.declare_dram_parameter("wv", [D, DH], bf16, isOutput=False)
    wo = nc.declare_dram_parameter("wo", [DH, D], bf16, isOutput=False)
    idents = nc.declare_dram_parameter("idents", [128, HH * 3 * 2 * 128], f8,
                                       isOutput=False)
    bq2 = nc.declare_dram_parameter("bq2", [128, 2], f32, isOutput=False)
    bo2 = nc.declare_dram_parameter("bo2", [128, 4], f32, isOutput=False)
    pad = nc.declare_dram_parameter("pad", [128, MT], f32, isOutput=False)
    pad4 = nc.declare_dram_parameter("pad4", [128, MT, HH], f32, isOutput=False)
    outT = nc.declare_dram_parameter("outT", [D, NSH], bf16, isOutput=True)

    with tile.TileContext(nc) as tc, ExitStack() as ctx:
        # fp32 matmul is 4 cycles/row on PE; fp32r streams at 1
        for _rep in range(reps):
            _run_once(nc, tc, ctx, tile, mybir, xtq, xtk, xtv, masksT,
                      wq, wk, wv, wo, idents, bq2, bo2, pad, pad4, outT)

    _split_matmul_waits(nc, mybir)
    return nc


def _run_once(nc, tc, ctx, tile, mybir, xtq, xtk, xtv, masksT, wq, wk, wv,
              wo, idents, bq2, bo2, pad, pad4, outT):
    from concourse.tile import add_dep_helper
    from contextlib import ExitStack

    f32 = mybir.dt.float32
    f32r = mybir.dt.float32r
    bf16 = mybir.dt.bfloat16
    f8 = mybir.dt.float8e4
    AFT = mybir.ActivationFunctionType
    ALU = mybir.AluOpType
    DR = mybir.MatmulPerfMode.DoubleRow
    mm = nc.tensor.matmul

    with ExitStack() as ctx:
        const_pool = ctx.enter_context(tc.tile_pool(name="const", bufs=1))
        persist = ctx.enter_context(tc.tile_pool(name="persist", bufs=1))

        loads = []
        # fp8 diag stationaries for the DoubleRow mask-bias matmuls:
        # per (local) head, 3 pairs (c0hi,c1hi), (c1lo,c2hi), (c0lo,c2lo)
        # matching the mask-slot pair slices
        id_sb = const_pool.tile([128, HH * 3, 2, 128], f8)
        ones_f = const_pool.tile([1, 64], f32)
        nc.vector.memset(ones_f[:], 1.0)
        ones_sb = const_pool.tile([1, 64], bf16)
        nc.vector.tensor_copy(ones_sb[:], ones_f[:])
        bq_sb = const_pool.tile([128, 2], f32)
        bo_sb = const_pool.tile([128, 4], f32)
        pad_sb = const_pool.tile([128, MT], f32)
        pad4_sb = const_pool.tile([128, MT, HH], f32)
        wo_sb = persist.tile([128, 2, D], bf16)

        # fp8 mask planes, 3 slots per key row (M0, M1, M2) -- consumed
        # directly as DoubleRow moving operands with pair slices [0:2],
        # [1:3] and the stride-2 [0:3:2] (no cast, no duplicate slot)
        mall = persist.tile([128, MT, 3, NSH], f8, name="mall")
        kT_sb = persist.tile([128, 2, N], f32r)
        qT_sb = persist.tile([128, 2, NSH], f32r)
        v_sb = persist.tile([128, MT, HH, C + 1], f32r)
        OT_sb = persist.tile([128, 2, NSH], bf16)
        outT_sb = persist.tile([128, 4, NSH], bf16)

        # ---- Phase A: projections ----
        with tc.tile_pool(name="xw", bufs=1) as xw_pool, \
             tc.tile_pool(name="psA", bufs=_B("PSA", 8), space="PSUM") as psA:
            wq_sb = xw_pool.tile([128, 4, DH], bf16, tag="w")
            wk_sb = xw_pool.tile([128, 4, DH], bf16, tag="w2")
            wv_sb = xw_pool.tile([128, 4, DH], bf16, tag="w3")
            xtq_sb = xw_pool.tile([128, 4, NSH], bf16, tag="xq")
            xtk_sb = xw_pool.tile([128, 4, N], bf16, tag="xk")
            xtv_sb = xw_pool.tile([128, 4, N], bf16, tag="xv")

            # q path first so the projections can start earliest; x_k / x_v
            # stream in column-group chunks matching the order the
            # projection loops consume them
            for ch in range(2):
                loads.append(nc.sync.dma_start(
                    wq_sb[:, 2 * ch:2 * ch + 2, :],
                    wq[ch * 256:(ch + 1) * 256, :].rearrange(
                        "(c p) d -> p c d", p=128)))
                loads.append(nc.sync.dma_start(
                    xtq_sb[:, 2 * ch:2 * ch + 2, 0:512],
                    xtq[ch * 256:(ch + 1) * 256, 0:512].rearrange(
                        "(c p) n -> p c n", p=128)))
            loads.append(nc.sync.dma_start(
                xtq_sb[:, :, 512:1024],
                xtq[:, 512:1024].rearrange("(c p) n -> p c n", p=128)))
            loads.append(nc.sync.dma_start(
                wk_sb[:], wk.rearrange("(c p) d -> p c d", p=128)))
            for mb in range(4):
                cs = slice(mb * 512, (mb + 1) * 512)
                loads.append(nc.sync.dma_start(
                    xtk_sb[:, :, cs],
                    xtk[:, cs].rearrange("(c p) n -> p c n", p=128)))
            xtk_last = loads[-1]
            loads.append(nc.sync.dma_start(bq_sb[:], bq2[:]))
            loads.append(nc.sync.dma_start(
                wv_sb[:], wv.rearrange("(c p) d -> p c d", p=128)))
            xtv_loads = []
            for mb in range(4):
                cs = slice(mb * 512, (mb + 1) * 512)
                xtv_loads.append(nc.sync.dma_start(
                    xtv_sb[:, :, cs],
                    xtv[:, cs].rearrange("(c p) n -> p c n", p=128)))
            loads.extend(xtv_loads)
            loads.append(nc.sync.dma_start(pad_sb[:], pad[:]))
            loads.append(nc.sync.dma_start(pad4_sb[:], pad4[:]))
            loads.append(nc.sync.dma_start(bo_sb[:], bo2[:]))

            # mask tiles + diag stationaries stream on the Pool engine's
            # SWDGE path (its own queue -- doesn't contend with the HWDGE
            # loads above); gated behind the xtk load so their transfers
            # don't steal DMA-engine bandwidth from the critical q/k path
            # (none of them is needed before phase B)
            pool_loads = [nc.gpsimd.dma_start(
                id_sb[:],
                idents.rearrange("p (i two m) -> p i two m", two=2, m=128))]
            for mt in range(3):
                pool_loads.append(nc.gpsimd.dma_start(
                    mall[:, mt, :, :],
                    masksT[mt * 128:(mt + 1) * 128, :].rearrange(
                        "p (s n) -> p s n", s=3)))
            for pl in pool_loads:
                add_dep_helper(pl.ins, xtk_last.ins, sync=True,
                               reason="delay pool prefetch past xtk load")

            # wo needed only in phase C -- load after the critical inputs
            loads.append(nc.sync.dma_start(
                wo_sb[:], wo.rearrange("(c p) d -> p c d", p=128)))

            nc.vector.tensor_copy(
                v_sb[:, :, :, C : C + 1],
                pad4_sb[:, :, :].rearrange("p m (h o) -> p m h o", o=1))

            # qT[dh, n] = (Wq/8).T @ xT_q  (+ bq/8 per-partition); nt-outer
            # so the second group does not wait for the next xtq chunk
            for nt in range(NT):
                for j in range(2):
                    ns = slice(nt * 512, (nt + 1) * 512)
                    ps = psA.tile([128, 512], f32, tag="psA")
                    for kc in range(4):
                        mm(ps[:], wq_sb[:, kc, j * 128:(j + 1) * 128],
                           xtq_sb[:, kc, ns], start=(kc == 0), stop=(kc == 3))
                    nc.scalar.activation(
                        qT_sb[:, j, ns], ps[:], AFT.Identity,
                        bias=bq_sb[:, j:j + 1])

            # kT[dh, m] = Wk.T @ xT_k   (k bias is softmax-invariant: dropped)
            for mb in range(4):
                for j in range(2):
                    ps = psA.tile([128, 512], f32, tag="psA")
                    for kc in range(4):
                        mm(ps[:], wk_sb[:, kc, j * 128:(j + 1) * 128],
                           xtk_sb[:, kc, mb * 512:(mb + 1) * 512],
                           start=(kc == 0), stop=(kc == 3))
                    if j == 0:
                        nc.scalar.copy(
                            kT_sb[:, j, mb * 512:(mb + 1) * 512], ps[:])
                    else:
                        nc.vector.tensor_copy(
                            kT_sb[:, j, mb * 512:(mb + 1) * 512], ps[:])

            # v[m, c] = xT_v.T @ Wv, padded key rows zeroed (scale by pad01)
            for mt in range(MT):
                ps = psA.tile([128, 512], f32, tag="psA")
                for kc in range(4):
                    mm(ps[:, 0:DH], xtv_sb[:, kc, mt * 128:(mt + 1) * 128],
                       wv_sb[:, kc, :], start=(kc == 0), stop=(kc == 3))
                if mt % 2 == 0:
                    nc.scalar.activation(
                        v_sb[:, mt, :, 0:C],
                        ps[:, 0:DH].rearrange("p (h c) -> p h c", h=HH),
                        AFT.Identity, scale=pad_sb[:, mt:mt + 1])
                else:
                    nc.vector.tensor_scalar(
                        v_sb[:, mt, :, 0:C],
                        ps[:, 0:DH].rearrange("p (h c) -> p h c", h=HH),
                        pad_sb[:, mt:mt + 1], None, ALU.mult)


        # PSUM pools for phase B: psO holds the 4 per-head output
        # accumulators, psS two 2-bank head-pair score tiles -- 8 banks
        psO = ctx.enter_context(tc.tile_pool(name="psO", bufs=4, space="PSUM"))
        psS = ctx.enter_context(tc.tile_pool(name="psS", bufs=_B("PSS", 2),
                                             space="PSUM"))
        pT_pool = ctx.enter_context(tc.tile_pool(name="pT", bufs=_B("PT", 4)))
        small_pool = ctx.enter_context(tc.tile_pool(name="small", bufs=8))

        # ---- Phase B: attention, one pass of 4 heads per query tile ----
        for nt in range(NT):
            ns = slice(nt * 512, (nt + 1) * 512)
            o_ps = [psO.tile([128, 512], f32, tag="psO", name=f"o_ps{nt}_{i}")
                    for i in range(4)]

            # software pipeline: the attn@V matmuls for key-tile mt-1 are
            # emitted between the score groups of tile mt, so PE never
            # waits on the current Exp (its pT is a full tile old)
            def emit_pv(pT, pmt, pr):
                for k in range(2):
                    l = 2 * pr + k
                    mm(o_ps[l][0:65, :], v_sb[:, pmt, l, :], pT[:, k, :],
                       start=(pmt == 0), stop=(pmt == MT - 1))

            prev = None
            for mt in range(MT):
                if nt == 0 and mt >= 3:
                    md = nc.gpsimd.dma_start(
                        mall[:, mt, :, :],
                        masksT[mt * 128:(mt + 1) * 128, :].rearrange(
                            "p (s n) -> p s n", s=3))
                    # pace each mask tile behind the xtv chunk whose
                    # v-tiles precede it, so mask transfers never starve
                    # the v path on the shared DMA engines
                    add_dep_helper(md.ins, xtv_loads[min(3, mt // 4)].ins,
                                   sync=True,
                                   reason="pace mask stream behind xtv")
                cur = []
                for pr in range(2):
                    # two heads share a 2-bank score tile so one Exp
                    # activation covers both (amortizes ACT overheads)
                    s_ps = psS.tile([128, 2, 512], f32, tag="psS")
                    for k in range(2):
                        l = 2 * pr + k
                        hj, ho = l // 2, (l % 2) * 64
                        mm(s_ps[:, k, :],
                           kT_sb[ho:ho + 64, hj, mt * 128:(mt + 1) * 128],
                           qT_sb[ho:ho + 64, hj, ns],
                           start=True, stop=False)
                        mm(s_ps[:, k, :], id_sb[:, l * 3 + 0, :, :],
                           mall[:, mt, 0:2, ns], start=False, stop=False,
                           perf_mode=DR)
                        mm(s_ps[:, k, :], id_sb[:, l * 3 + 1, :, :],
                           mall[:, mt, 1:3, ns], start=False, stop=False,
                           perf_mode=DR)
                        mm(s_ps[:, k, :], id_sb[:, l * 3 + 2, :, :],
                           mall[:, mt, 0:3:2, ns], start=False, stop=True,
                           perf_mode=DR)
                    pT = pT_pool.tile([128, 2, 512], f32r, tag="pT")
                    nc.scalar.activation(pT[:], s_ps[:], AFT.Exp)
                    cur.append(pT)
                    if prev is not None:
                        emit_pv(prev[pr], mt - 1, pr)
                prev = cur

            # normalize: OT[h-rows, n] = o[c, n] / rowsum[n].  Per head:
            # reciprocal (DVE) -> bf16 rounding copy (Pool) -> the o rows
            # evacuate to SBUF on ACT -> both reciprocal rows of a 2-head
            # block are broadcast by K=1 matmuls into the two partition
            # halves of the even head's (now-evacuated) o_ps bank -> ONE
            # [128,512] multiply per 2-head block (DVE).  The final attn@V
            # matmuls interleave so recips start as soon as their head's
            # accumulation closes.
            for g in range(2):
                emit_pv(prev[g], MT - 1, g)
            osbp = []
            for g in range(2):
                op = small_pool.tile([128, 512], f32, tag="osb",
                                     name=f"osb{nt}_{g}")
                osbp.append(op)
                for k in range(2):
                    i = 2 * g + k
                    rsb = small_pool.tile([1, 512], f32, tag="rsb",
                                          name=f"rsb{nt}_{i}")
                    nc.vector.reciprocal(rsb[:], o_ps[i][64:65, :])
                    rsr = small_pool.tile([1, 512], bf16, tag="rsr",
                                          name=f"rsr{nt}_{i}")
                    nc.gpsimd.tensor_copy(rsr[:], rsb[:])
                    nc.scalar.copy(op[64 * k:64 * k + 64, :],
                                   o_ps[i][0:64, :])
                    mm(o_ps[2 * g][64 * k:64 * k + 64, :], ones_sb[0:1, :],
                       rsr[0:1, :], start=True, stop=True,
                       skip_group_check=True)
            for i in range(4):
                g, k = i // 2, i % 2
                hj, ho = i // 2, (i % 2) * 64
                nc.vector.tensor_tensor(
                    OT_sb[ho:ho + 64, hj, ns],
                    osbp[g][64 * k:64 * k + 64, :],
                    o_ps[2 * g][64 * k:64 * k + 64, :], ALU.mult)

            # ---- Phase C for this query tile: partial output projection
            # (host sums the two head-halves). Emitted inside the nt loop
            # in psO-tag banks, so the nt0 projection overlaps the nt1
            # attention loop and the nt1 projection waits only on the
            # normalization it needs anyway.
            o_pss = [psO.tile([128, 512], f32, tag="psO",
                              name=f"opsC{nt}_{j}") for j in range(4)]
            for gi in range(2):
                for jt in range(4):
                    mm(o_pss[jt][:],
                       wo_sb[:, gi, jt * 128:(jt + 1) * 128],
                       OT_sb[:, gi, ns], start=(gi == 0), stop=(gi == 1))
            # bias adds alternate ACT/DVE so both halves finish ~together;
            # two half stores so the first transfer overlaps the second half
            for jt in range(4):
                if jt % 2 == 0:
                    nc.scalar.activation(outT_sb[:, jt, ns],
                                         o_pss[jt][:],
                                         AFT.Identity,
                                         bias=bo_sb[:, jt:jt + 1])
                else:
                    nc.vector.tensor_scalar(outT_sb[:, jt, ns],
                                            o_pss[jt][:],
                                            bo_sb[:, jt:jt + 1], None,
                                            ALU.add)
                if jt % 2 == 1:
                    nc.sync.dma_start(
                        outT[(jt - 1) * 128:(jt + 1) * 128, ns].rearrange(
                            "(c p) n -> p c n", p=128),
                        outT_sb[:, jt - 1:jt + 1, ns])


# every TPB instruction encoding in this walrus build tolerates only a
# single semaphore wait -- split extras regardless of opcode
_NO_SPLIT_TYPES = {"InstEventSemaphore"}


def _split_matmul_waits(nc, mybir):
    """Several engine instruction encodings tolerate only one semaphore
    wait; move extra waits onto standalone single-wait EventSemaphore
    instructions inserted right before them on the same engine queue."""
    import bass_rust

    n = 0
    for bb in nc.m.functions[0].blocks:
        insts = list(bb.instructions)
        out = []
        changed = False
        for i in insts:
            si = i.sync_info
            if (type(i).__name__ not in _NO_SPLIT_TYPES and si is not None
                    and len(si.on_wait) > 1):
                w = list(si.on_wait)
                for wx in w[:-1]:
                    ev = mybir.InstEventSemaphore(name=f"mmw_{n}_{i.name}",
                                                  ins=[], outs=[])
                    ev.engine = i.engine
                    ev.sync_info = bass_rust.SyncInfo(on_wait=[wx],
                                                      on_update=[])
                    out.append(ev)
                    n += 1
                si.on_wait = [w[-1]]
                changed = True
            out.append(i)
        if changed:
            bb.instructions = out


def _host_prep(inputs):
    x_q = np.asarray(inputs["x_q"], np.float32)
    x_k = np.asarray(inputs["x_k"], np.float32)
    x_v = np.asarray(inputs["x_v"], np.float32)
    attn_mask = np.asarray(inputs["attn_mask"]).astype(np.uint8)
    kpm = np.asarray(inputs["key_padding_mask"]).astype(bool)
    Wq = np.asarray(inputs["Wq"], np.float32)
    Wk = np.asarray(inputs["Wk"], np.float32)
    Wv = np.asarray(inputs["Wv"], np.float32)
    Wo = np.asarray(inputs["Wo"], np.float32)
    bq = np.asarray(inputs["bq"], np.float32)
    bv = np.asarray(inputs["bv"], np.float32)
    bo = np.asarray(inputs["bo"], np.float32)
    mw = np.asarray(inputs["mask_weight"], np.float64)

    # c[h,r] = softmax(mask_weight[h,:R]) * mask_weight[h,R]
    e = np.exp(mw[:, :R] - mw[:, :R].max(axis=1, keepdims=True))
    w = e / e.sum(axis=1, keepdims=True)
    c = w * mw[:, R:R + 1]                               # [H, R] float64

    # split each coefficient into hi+lo fp8 terms (max residual ~7.5e-3)
    # so the bias matmuls can run in fp8 DoubleRow mode (0.5 PE cycles/row,
    # two mask planes per matmul)
    f8 = ml_dtypes.float8_e4m3
    chi = c.astype(f8).astype(np.float64)
    clo = (c - chi).astype(f8).astype(np.float64)

    # per head, 3 stationary diag-pairs: (c0hi,c1hi), (c0lo,c1lo), (c2hi,c2lo)
    pairs = np.zeros((H, 3, 2), np.float64)
    pairs[:, 0, 0] = chi[:, 0]
    pairs[:, 0, 1] = chi[:, 1]
    pairs[:, 1, 0] = clo[:, 1]
    pairs[:, 1, 1] = chi[:, 2]
    pairs[:, 2, 0] = clo[:, 0]
    pairs[:, 2, 1] = clo[:, 2]
    eye = np.eye(128, dtype=np.float64)
    idents_hh = []
    for hh in range(2):
        idents = np.zeros((HH * 3 * 2, 128, 128), np.float64)
        for l in range(HH):
            h = hh * HH + l
            for p in range(3):
                for t in range(2):
                    idents[(l * 3 + p) * 2 + t] = eye * (-pairs[h, p, t])
        # partition-major so the DMA is one contiguous descriptor per row
        idents_hh.append(np.ascontiguousarray(
            idents.transpose(1, 0, 2)).reshape(128, HH * 3 * 2 * 128)
            .astype(f8))

    scale = np.float32(1.0 / np.sqrt(C))
    wq_s = (Wq * scale).astype(np.float32)
    bq_s = (bq * scale).astype(np.float32)
    bo_p = (bv @ Wo + bo).astype(np.float32)

    bf = ml_dtypes.bfloat16
    in_maps = []
    for core in range(NCORES):
        b, nh, hh = core // 4, (core // 2) % 2, core % 2
        n0 = nh * NSH
        ds = slice(hh * DH, (hh + 1) * DH)
        pad01 = (~kpm[b]).astype(np.float32)             # [N]
        pad2 = np.ascontiguousarray(pad01.reshape(MT, 128).T)
        pad4 = np.ascontiguousarray(np.repeat(pad2[:, :, None], HH, axis=2))
        bo_half = bo_p if hh == 0 else np.zeros_like(bo_p)
        m = dict(
            wq=np.ascontiguousarray(wq_s[:, ds]).astype(bf),
            wk=np.ascontiguousarray(Wk[:, ds]).astype(bf),
            wv=np.ascontiguousarray(Wv[:, ds]).astype(bf),
            wo=np.ascontiguousarray(Wo[ds, :]).astype(bf),
            idents=idents_hh[hh],
            bq2=np.ascontiguousarray(bq_s[ds].reshape(2, 128).T),
            bo2=np.ascontiguousarray(bo_half.reshape(4, 128).T),
            pad=pad2,
            pad4=pad4,
        )
        m["xtq"] = np.ascontiguousarray(x_q[n0:n0 + NSH, b, :].T).astype(bf)
        m["xtk"] = np.ascontiguousarray(x_k[:, b, :].T).astype(bf)
        m["xtv"] = np.ascontiguousarray(x_v[:, b, :].T).astype(bf)
        # [N, 4, NSH] fp8 slots (M0, M1, M2, M2) for the DoubleRow pairs
        planes = attn_mask[b, :, n0:n0 + NSH, :].transpose(0, 2, 1)
        m4 = np.empty((N, 3, NSH), f8)
        for r in range(R):
            m4[:, r, :] = planes[r]
        m["masksT"] = np.ascontiguousarray(m4.reshape(N, 3 * NSH))
        in_maps.append(m)
    return in_maps


def kernel(**inputs) -> np.ndarray:
    from concourse.bass_utils import run_bass_kernel_spmd

    if "nc" not in _cache:
        _cache["nc"] = _build_program()
    nc = _cache["nc"]

    in_maps = _host_prep(inputs)
    res = run_bass_kernel_spmd(nc, in_maps, list(range(NCORES)))

    out = np.zeros((N, B, D), np.float32)
    for core in range(NCORES):
        b, nh, hh = core // 4, (core // 2) % 2, core % 2
        n0 = nh * NSH
        out[n0:n0 + NSH, b, :] += res.results[core]["outT"].T.astype(np.float32)
    return out
